# revision 20
# baseline (speedup 1.0000x reference)
import sys

sys.path.insert(0, "/opt/trn_rl_repo")

from contextlib import ExitStack

import ml_dtypes
import numpy as np

from concourse import bass, mybir, tile
from concourse.bass_utils import run_bass_kernel_spmd
from concourse.vector_clock import ScopedClock


def _patched_drain_and_barrier(self, tick_clock, wait_clock):
    # Workaround: this compiler rejects a drain carrying >1 sem wait
    # ([NCC_INLA001]); split extra waits onto single-wait nops.
    drain_inst = self.nc.sync.drain()
    wait_clock.add_sem_waits(
        drain_inst.ins, ScopedClock({None: tick_clock.global_clock})
    )
    si = drain_inst.ins.sync_info
    waits = list(si.on_wait) if si and si.on_wait else []
    if len(waits) > 1:
        drain_inst.ins.sync_info = mybir.SyncInfo(
            on_wait=[waits[0]], on_update=list(si.on_update or [])
        )
        for w in waits[1:]:
            nop = self.nc.sync.nop(nofuse=True)
            nop.ins.sync_info = mybir.SyncInfo(on_wait=[w], on_update=[])
    self.nc.all_engine_barrier()
    popped = self.nc._tile_sem_poison_stack.pop()
    assert popped is self._sem_poison
    self.nc.clear_and_free_semaphores(list(self.sems.allocated().values()))
    self.nc.all_engine_barrier()


tile.TileContext._drain_and_barrier = _patched_drain_and_barrier


def _split_excess_waits(nc, limit=1):
    # Workaround: this compiler allows only one sem wait on several
    # instruction encodings; move extra waits onto same-engine nops.
    eng_map = {
        mybir.EngineType.PE: nc.tensor,
        mybir.EngineType.Activation: nc.scalar,
        mybir.EngineType.DVE: nc.vector,
        mybir.EngineType.Pool: nc.gpsimd,
        mybir.EngineType.SP: nc.sync,
    }
    for blk in nc.cur_f.blocks:
        orig = list(blk.instructions)
        out = []
        for ins in orig:
            si = ins.sync_info
            waits = list(si.on_wait) if si and si.on_wait else []
            eng = eng_map.get(ins.engine)
            if len(waits) > limit and eng is not None:
                extra, keep = waits[:-limit], waits[-limit:]
                for w in extra:
                    nop = eng.nop(nofuse=True).ins
                    tail = nc.cur_f.blocks[-1].instructions
                    assert tail[-1] is nop
                    tail.pop()
                    nop.sync_info = mybir.SyncInfo(on_wait=[w], on_update=[])
                    out.append(nop)
                ins.sync_info = mybir.SyncInfo(
                    on_wait=keep, on_update=list(si.on_update or [])
                )
            out.append(ins)
        blk.instructions[:] = out

bf16 = ml_dtypes.bfloat16
BF = bass.mybir.dt.bfloat16
F32 = bass.mybir.dt.float32
AF = mybir.ActivationFunctionType
ALU = mybir.AluOpType

B, S, E, H, D = 2, 2048, 2048, 16, 128
BS = B * S
NCORES = 8
HPC = H // NCORES  # heads per core
DC = HPC * D  # per-core head-dim width (256)
SCALE = 1.0 / float(np.sqrt(D))
LAG = 4  # ctx matmul lags sc matmul by this many k-chunks

TRACE = False
LAST_RESULTS = None
_NC_CACHE = None


def _build():
    nc = bass.Bass()
    # xP: slice-major packed x — row j*128+p holds x[t*128+p, j*512+c] at
    # col t*512+c, so one DMA per 512-token slice reads 16KB-contiguous rows
    xP = nc.declare_dram_parameter("xP", (8 * 128, 16 * 512), BF, isOutput=False)
    # weights packed so SBUF partition p's row is contiguous in DRAM
    wqP = nc.declare_dram_parameter("wqP", (128, 16 * DC), BF, isOutput=False)
    wkP = nc.declare_dram_parameter("wkP", (128, 16 * DC), BF, isOutput=False)
    wvP = nc.declare_dram_parameter("wvP", (128, 16 * DC), BF, isOutput=False)
    woP = nc.declare_dram_parameter("woP", (128, HPC * E), BF, isOutput=False)
    bqd = nc.declare_dram_parameter("bq", (DC, 1), F32, isOutput=False)
    bkd = nc.declare_dram_parameter("bk", (DC, 1), F32, isOutput=False)
    trid = nc.declare_dram_parameter("tri", (128, 128), BF, isOutput=False)
    onkd = nc.declare_dram_parameter("onesk", (128, 128), BF, isOutput=False)
    yd = nc.declare_dram_parameter("y", (BS, E), BF, isOutput=True)

    with ExitStack() as ctx:
        tc = ctx.enter_context(tile.TileContext(nc))
        wp = ctx.enter_context(tc.tile_pool(name="wp", bufs=1))
        xp = ctx.enter_context(tc.tile_pool(name="xp", bufs=4))
        bp = ctx.enter_context(tc.tile_pool(name="bp", bufs=2))
        cp = ctx.enter_context(tc.tile_pool(name="cp", bufs=1))
        pp = ctx.enter_context(tc.tile_pool(name="pp", bufs=6))
        dp = ctx.enter_context(tc.tile_pool(name="dp", bufs=2))
        yp = ctx.enter_context(tc.tile_pool(name="yp", bufs=3))
        ps = ctx.enter_context(tc.tile_pool(name="ps", bufs=1, space="PSUM"))

        wq_sb = wp.tile([128, 16, DC], BF)
        wk_sb = wp.tile([128, 16, DC], BF)
        wv_sb = wp.tile([128, 16, DC], BF)
        wo_sb = wp.tile([128, HPC, E], BF)
        bq_sb = wp.tile([128, HPC, 1], F32)
        bk_sb = wp.tile([128, HPC, 1], F32)
        tri_sb = wp.tile([128, 128], BF)
        onk_sb = wp.tile([128, 128], BF)

        x_tiles = {}

        def emit_xload(b, j, split=1):
            x_sb = xp.tile([128, 16, 512], BF, tag="x", name=f"x{b}{j}")
            r0 = (b * 4 + j) * 128
            cper = 8192 // split
            for u in range(split):
                nc.sync.dma_start(
                    x_sb[:, u * (16 // split) : (u + 1) * (16 // split), :],
                    xP[r0 : r0 + 128, u * cper : (u + 1) * cper],
                )
            x_tiles[(b, j)] = x_sb

        # x slice 0 on the Sync DMA queue; weights in parallel on the
        # Scalar engine's queue so the first projection starts ~11us in
        emit_xload(0, 0, split=4)
        for u in range(4):
            nc.scalar.dma_start(
                wq_sb[:, u * 4 : (u + 1) * 4, :],
                wqP[:, u * 4 * DC : (u + 1) * 4 * DC],
            )
        nc.scalar.dma_start(wk_sb[:], wkP[:])
        # wv on the (faster) Sync queue right behind x slice 0
        nc.sync.dma_start(wv_sb[:], wvP[:])
        for h in range(HPC):
            nc.scalar.dma_start(bq_sb[:, h, :], bqd[h * 128 : (h + 1) * 128, :])
            nc.scalar.dma_start(bk_sb[:, h, :], bkd[h * 128 : (h + 1) * 128, :])
        nc.scalar.dma_start(tri_sb[:], trid[:])
        nc.scalar.dma_start(onk_sb[:], onkd[:])

        # per-batch attention tensors (double-buffered across batches)
        qT_tiles, kT_tiles, v_tiles, cN_tiles = {}, {}, {}, {}

        def emit_proj(b, j, bg=None, hook=None):
            if b not in qT_tiles:
                qT_tiles[b] = bp.tile([128, HPC, S], BF, tag="qT", name=f"qT{b}")
                kT_tiles[b] = bp.tile([128, HPC, S], BF, tag="kT", name=f"kT{b}")
                v_tiles[b] = bp.tile([128, 16, DC], BF, tag="v", name=f"v{b}")
            x_sb = x_tiles[(b, j)]
            qT_sb, kT_sb, v_sb = qT_tiles[b], kT_tiles[b], v_tiles[b]
            js = slice(j * 512, (j + 1) * 512)
            first = True
            for w_sb, b_sb, o_sb in (
                (wq_sb, bq_sb, qT_sb),
                (wk_sb, bk_sb, kT_sb),
            ):
                for m in range(HPC):
                    p_ps = ps.tile([128, 512], F32, tag="pr", bufs=2)
                    for t in range(16):
                        nc.tensor.matmul(
                            p_ps[:],
                            w_sb[:, t, m * 128 : (m + 1) * 128],
                            x_sb[:, t, :],
                            start=(t == 0),
                            stop=(t == 15),
                        )
                    nc.scalar.activation(
                        o_sb[:, m, js], p_ps[:], AF.Identity, bias=b_sb[:, m, :]
                    )
                    if first:
                        # boundary work (prev batch's tails + den flush)
                        # lands here so the Q group hides its latency
                        if hook is not None:
                            hook()
                        first = False
                    elif bg is not None:
                        next(bg, None)
            for si in range(4):
                v_ps = ps.tile([128, DC], F32, tag="pr", bufs=2)
                for t in range(16):
                    nc.tensor.matmul(
                        v_ps[:],
                        x_sb[:, t, si * 128 : (si + 1) * 128],
                        wv_sb[:, t, :],
                        start=(t == 0),
                        stop=(t == 15),
                    )
                nc.vector.tensor_copy(v_sb[:, j * 4 + si, :], v_ps[:])
                if bg is not None:
                    next(bg, None)

        # deferred denominator chains + cross-group ctx-matmul tails: both
        # are emitted inside the NEXT head group so the PE never idles on
        # this group's exp/accumulate latency
        pending = []
        tail_q = []

        def flush_pending(n=None):
            cnt = len(pending) if n is None else min(n, len(pending))
            for _ in range(cnt):
                b, qb, h, pacc_bf, ctx_ps = pending.pop(0)
                qs = slice(qb * 512, (qb + 1) * 512)
                den_ps = ps.tile([128, 512], F32, tag="pr", bufs=2)
                nc.tensor.matmul(
                    den_ps[:], onk_sb[:], pacc_bf[:], start=True, stop=True
                )
                lnd_sb = dp.tile([128, 512], F32, tag="lnd", bufs=2)
                nc.scalar.activation(lnd_sb[:], den_ps[:], AF.Ln)
                recb_sb = dp.tile([128, 512], F32, tag="recb", bufs=2)
                nc.scalar.activation(recb_sb[:], lnd_sb[:], AF.Exp, scale=-1.0)
                nc.vector.tensor_tensor(
                    cN_tiles[b][:, h, qs], ctx_ps[:], recb_sb[:], ALU.mult
                )

        def drain_tails(n=None):
            cnt = len(tail_q) if n is None else min(n, len(tail_q))
            for _ in range(cnt):
                tail_q.pop(0)()

        def emit_attn(b, qb, h, bg=None, bg_from=3):
            if b not in cN_tiles:
                cN_tiles[b] = cp.tile([128, HPC, S], BF, tag="cN", name=f"cN{b}")
            qT_sb, kT_sb, v_sb = qT_tiles[b], kT_tiles[b], v_tiles[b]
            q0 = qb * 512
            kmax = 4 * qb + 4
            flush_at = min(4, kmax - 1)
            pts = []
            ctx_ps = ps.tile([128, 512], F32, tag="cx", bufs=2)
            # exp-sum accumulators: even k-chunks on DVE, odd on GpSimd,
            # combined (and cast to bf16) at the end
            pacc_a = dp.tile([128, 512], F32, tag="pacc_a", bufs=2)
            pacc_b = dp.tile([128, 512], F32, tag="pacc_b", bufs=2)
            pacc_bf = dp.tile([128, 512], BF, tag="paccb", bufs=2)
            if qb == 0:
                # odd accumulator's first write only covers [128:512]
                nc.vector.memzero(pacc_b[:, 0:128])

            def ctx_mm(jj):
                p_t, slot, sl = pts[jj]
                nc.tensor.matmul(
                    ctx_ps[:, sl],
                    v_sb[:, jj, h * 128 : (h + 1) * 128],
                    p_t[:, slot, sl],
                    start=(jj == 0),
                    stop=(jj == kmax - 1),
                )

            ctx_next = 0
            for kc in range(0, kmax, 2):
                # previous group's ctx tail while its final exps finish
                drain_tails(2 if kc == 0 else 3)
                if kc <= flush_at <= kc + 1:
                    flush_pending(1)
                # two score chunks share a 2-bank psum tile and ONE exp —
                # halves the per-ACTIVATE fixed overhead on the Scalar engine
                sc_pr = ps.tile([128, 2, 512], F32, tag="sc", bufs=2, name="sc_pr")
                p_t = pp.tile([128, 2, 512], BF, name="p_t")
                sls = []
                for i in range(2):
                    r = kc + i - 4 * qb  # >=0 on the diagonal band
                    sl = slice(r * 128, 512) if r > 0 else slice(0, 512)
                    sls.append((r, sl))
                    nc.tensor.matmul(
                        sc_pr[:, i, sl],
                        kT_sb[:, h, (kc + i) * 128 : (kc + i + 1) * 128],
                        qT_sb[:, h, q0 + sl.start : q0 + 512],
                        start=True,
                        stop=True,
                    )
                # full-pair exp: unused regions hold stale-but-finite scores
                # and are never read downstream
                nc.scalar.activation(p_t[:, :, :], sc_pr[:, :, :], AF.Exp)
                for i in range(2):
                    r, sl = sls[i]
                    if r >= 0:
                        nc.vector.tensor_tensor(
                            p_t[:, i, r * 128 : (r + 1) * 128],
                            p_t[:, i, r * 128 : (r + 1) * 128],
                            tri_sb[:],
                            ALU.mult,
                        )
                    eng, pacc = (
                        (nc.gpsimd, pacc_a) if i == 0 else (nc.vector, pacc_b)
                    )
                    if kc < 2:
                        eng.tensor_copy(pacc[:, sl], p_t[:, i, sl])
                    else:
                        eng.tensor_tensor(
                            pacc[:, sl], pacc[:, sl], p_t[:, i, sl], ALU.add
                        )
                    pts.append((p_t, i, sl))
                while ctx_next <= kc + 1 - LAG:
                    ctx_mm(ctx_next)
                    ctx_next += 1
                if bg is not None:
                    for i in range(2):
                        if kc + i >= bg_from:
                            next(bg, None)

            def tail(jj):
                return lambda: ctx_mm(jj)

            for jj in range(ctx_next, kmax):
                tail_q.append(tail(jj))

            nc.vector.tensor_tensor(pacc_bf[:], pacc_a[:], pacc_b[:], ALU.add)
            pending.append((b, qb, h, pacc_bf, ctx_ps))

        def outproj_gen(b, qb):
            # one yield per (2 matmuls + psum evacuation); interleaved into
            # the next q-block's attention so evac latency hides behind MMs
            cN_sb = cN_tiles[b]
            for qc in range(4):
                r0 = b * S + qb * 512 + qc * 128
                c0 = qb * 512 + qc * 128
                for e2 in range(2):
                    y_sb = yp.tile([128, 1024], BF, tag="y", name="y_sb")
                    for half in range(2):
                        eb = e2 * 1024 + half * 512
                        y_ps = ps.tile([128, 512], F32, tag="pr", bufs=2)
                        nc.tensor.matmul(
                            y_ps[:],
                            cN_sb[:, 0, c0 : c0 + 128],
                            wo_sb[:, 0, eb : eb + 512],
                            start=True,
                            stop=False,
                        )
                        nc.tensor.matmul(
                            y_ps[:],
                            cN_sb[:, 1, c0 : c0 + 128],
                            wo_sb[:, 1, eb : eb + 512],
                            start=False,
                            stop=True,
                        )
                        if half == 0:
                            nc.vector.tensor_copy(y_sb[:, :512], y_ps[:])
                        else:
                            nc.scalar.copy(y_sb[:, 512:], y_ps[:])
                        yield
                    nc.sync.dma_start(
                        yd[r0 : r0 + 128, e2 * 1024 : (e2 + 1) * 1024], y_sb[:]
                    )

        def drain(gen):
            if gen is not None:
                for _ in gen:
                    pass

        carry = None  # outproj generator carried from the previous batch
        for b in range(B):
            def boundary():
                drain_tails()
                flush_pending()

            for j in range(4):
                if b == 0 and j > 0:
                    emit_xload(b, j)
                emit_proj(
                    b,
                    j,
                    bg=carry if j == 0 else None,
                    hook=boundary if j == 0 else None,
                )
                if j == 0:
                    drain(carry)
                    carry = None
            if b == 0:
                # wo is first needed by outproj pulls ~100us in; x prefetch
                # for the next batch ahead of this batch's y writeback
                nc.sync.dma_start(wo_sb[:], woP[:])
                for j in range(4):
                    emit_xload(b + 1, j)
            bg = None
            for qb in (range(3, -1, -1) if b == 0 else range(4)):
                # bg units read cN for both heads of qb-1, whose h1 flush
                # happens inside this qb's h0 group at chunk<=5 — pull in
                # h0 only from chunk 6, freely in h1
                emit_attn(b, qb, 0, bg=bg, bg_from=5)
                emit_attn(b, qb, 1, bg=bg, bg_from=3)
                drain(bg)
                bg = outproj_gen(b, qb)
            carry = bg
        drain_tails()
        flush_pending()
        drain(carry)
    _split_excess_waits(nc)
    return nc


def _tri_np():
    kk = np.arange(128)[:, None]
    qq = np.arange(128)[None, :]
    return np.where(kk > qq, 0.0, 1.0).astype(bf16)


def _pack_x(xT):
    # xP[j*128+p, t*512+c] = xT[t*128+p, j*512+c]
    x4 = xT.reshape(16, 128, 8, 512)
    return np.ascontiguousarray(x4.transpose(2, 1, 0, 3).reshape(8 * 128, 8192))


def _pack_w(wT):
    # wP[p, t*C+c] = wT[t*128+p, c]
    t, c = wT.shape[0] // 128, wT.shape[1]
    w3 = wT.reshape(t, 128, c)
    return np.ascontiguousarray(w3.transpose(1, 0, 2).reshape(128, t * c))


def kernel(**inputs):
    global LAST_RESULTS, _NC_CACHE
    x = np.asarray(inputs["x"], np.float32)
    Wq = np.asarray(inputs["Wq"], np.float32)
    bq = np.asarray(inputs["bq"], np.float32)
    Wk = np.asarray(inputs["Wk"], np.float32)
    bk = np.asarray(inputs["bk"], np.float32)
    Wv = np.asarray(inputs["Wv"], np.float32)
    bv = np.asarray(inputs["bv"], np.float32)
    Wo = np.asarray(inputs["Wo"], np.float32)
    bo = np.asarray(inputs["bo"], np.float32)

    xT = np.ascontiguousarray(x.reshape(BS, E).T).astype(bf16)
    xPk = _pack_x(xT)
    tri = _tri_np()
    onesk = np.ones((128, 128), bf16)

    in_maps = []
    for c in range(NCORES):
        dsl = slice(c * DC, (c + 1) * DC)
        in_maps.append(
            {
                "xP": xPk,
                "wqP": _pack_w((Wq[dsl].T * SCALE).astype(bf16)),
                "wkP": _pack_w(Wk[dsl].T.astype(bf16)),
                "wvP": _pack_w(Wv[dsl].T.astype(bf16)),
                "woP": _pack_w(Wo[:, dsl].T.astype(bf16)),
                "bq": np.ascontiguousarray((bq[dsl] * SCALE).reshape(DC, 1)),
                "bk": np.ascontiguousarray(bk[dsl].reshape(DC, 1)),
                "tri": tri,
                "onesk": onesk,
            }
        )

    if _NC_CACHE is None:
        _NC_CACHE = _build()
    res = run_bass_kernel_spmd(_NC_CACHE, in_maps, core_ids=list(range(NCORES)), trace=TRACE)
    LAST_RESULTS = res

    acc = None
    for r in res.results:
        yc = np.asarray(r["y"], np.float32)
        acc = yc if acc is None else acc + yc
    bo_eff = bo + bv @ Wo.T
    acc += bo_eff[None, :]
    return acc.reshape(B, S, E).astype(np.float32)


# revision 21
# speedup vs baseline: 1.0701x; 1.0701x over previous
import sys

sys.path.insert(0, "/opt/trn_rl_repo")

from contextlib import ExitStack

import ml_dtypes
import numpy as np

from concourse import bass, mybir, tile
from concourse.bass_utils import run_bass_kernel_spmd
from concourse.vector_clock import ScopedClock


def _patched_drain_and_barrier(self, tick_clock, wait_clock):
    # Workaround: this compiler rejects a drain carrying >1 sem wait
    # ([NCC_INLA001]); split extra waits onto single-wait nops.
    drain_inst = self.nc.sync.drain()
    wait_clock.add_sem_waits(
        drain_inst.ins, ScopedClock({None: tick_clock.global_clock})
    )
    si = drain_inst.ins.sync_info
    waits = list(si.on_wait) if si and si.on_wait else []
    if len(waits) > 1:
        drain_inst.ins.sync_info = mybir.SyncInfo(
            on_wait=[waits[0]], on_update=list(si.on_update or [])
        )
        for w in waits[1:]:
            nop = self.nc.sync.nop(nofuse=True)
            nop.ins.sync_info = mybir.SyncInfo(on_wait=[w], on_update=[])
    self.nc.all_engine_barrier()
    popped = self.nc._tile_sem_poison_stack.pop()
    assert popped is self._sem_poison
    self.nc.clear_and_free_semaphores(list(self.sems.allocated().values()))
    self.nc.all_engine_barrier()


tile.TileContext._drain_and_barrier = _patched_drain_and_barrier


def _split_excess_waits(nc, limit=1):
    # Workaround: this compiler allows only one sem wait on several
    # instruction encodings; move extra waits onto same-engine nops.
    eng_map = {
        mybir.EngineType.PE: nc.tensor,
        mybir.EngineType.Activation: nc.scalar,
        mybir.EngineType.DVE: nc.vector,
        mybir.EngineType.Pool: nc.gpsimd,
        mybir.EngineType.SP: nc.sync,
    }
    for blk in nc.cur_f.blocks:
        orig = list(blk.instructions)
        out = []
        for ins in orig:
            si = ins.sync_info
            waits = list(si.on_wait) if si and si.on_wait else []
            eng = eng_map.get(ins.engine)
            if len(waits) > limit and eng is not None:
                extra, keep = waits[:-limit], waits[-limit:]
                for w in extra:
                    nop = eng.nop(nofuse=True).ins
                    tail = nc.cur_f.blocks[-1].instructions
                    assert tail[-1] is nop
                    tail.pop()
                    nop.sync_info = mybir.SyncInfo(on_wait=[w], on_update=[])
                    out.append(nop)
                ins.sync_info = mybir.SyncInfo(
                    on_wait=keep, on_update=list(si.on_update or [])
                )
            out.append(ins)
        blk.instructions[:] = out

bf16 = ml_dtypes.bfloat16
BF = bass.mybir.dt.bfloat16
F32 = bass.mybir.dt.float32
AF = mybir.ActivationFunctionType
ALU = mybir.AluOpType

B, S, E, H, D = 2, 2048, 2048, 16, 128
BS = B * S
NCORES = 8
HPC = H // NCORES  # heads per core
DC = HPC * D  # per-core head-dim width (256)
SCALE = 1.0 / float(np.sqrt(D))
LAG = 4  # ctx matmul lags sc matmul by this many k-chunks

TRACE = False
LAST_RESULTS = None
_NC_CACHE = None


def _build():
    nc = bass.Bass()
    # xP: slice-major packed x — row j*128+p holds x[t*128+p, j*512+c] at
    # col t*512+c, so one DMA per 512-token slice reads 16KB-contiguous rows
    xP = nc.declare_dram_parameter("xP", (8 * 128, 16 * 512), BF, isOutput=False)
    # weights packed so SBUF partition p's row is contiguous in DRAM
    wqP = nc.declare_dram_parameter("wqP", (128, 16 * DC), BF, isOutput=False)
    wkP = nc.declare_dram_parameter("wkP", (128, 16 * DC), BF, isOutput=False)
    wvP = nc.declare_dram_parameter("wvP", (128, 16 * DC), BF, isOutput=False)
    woP = nc.declare_dram_parameter("woP", (128, HPC * E), BF, isOutput=False)
    bqd = nc.declare_dram_parameter("bq", (DC, 1), F32, isOutput=False)
    bkd = nc.declare_dram_parameter("bk", (DC, 1), F32, isOutput=False)
    trid = nc.declare_dram_parameter("tri", (128, 128), BF, isOutput=False)
    onkd = nc.declare_dram_parameter("onesk", (128, 128), BF, isOutput=False)
    yd = nc.declare_dram_parameter("y", (BS, E), BF, isOutput=True)

    with ExitStack() as ctx:
        tc = ctx.enter_context(tile.TileContext(nc))
        wp = ctx.enter_context(tc.tile_pool(name="wp", bufs=1))
        xp = ctx.enter_context(tc.tile_pool(name="xp", bufs=4))
        bp = ctx.enter_context(tc.tile_pool(name="bp", bufs=2))
        cp = ctx.enter_context(tc.tile_pool(name="cp", bufs=1))
        pp = ctx.enter_context(tc.tile_pool(name="pp", bufs=6))
        dp = ctx.enter_context(tc.tile_pool(name="dp", bufs=2))
        yp = ctx.enter_context(tc.tile_pool(name="yp", bufs=3))
        ps = ctx.enter_context(tc.tile_pool(name="ps", bufs=1, space="PSUM"))

        wq_sb = wp.tile([128, 16, DC], BF)
        wk_sb = wp.tile([128, 16, DC], BF)
        wv_sb = wp.tile([128, 16, DC], BF)
        wo_sb = wp.tile([128, HPC, E], BF)
        bq_sb = wp.tile([128, HPC, 1], F32)
        bk_sb = wp.tile([128, HPC, 1], F32)
        tri_sb = wp.tile([128, 128], BF)
        onk_sb = wp.tile([128, 128], BF)

        x_tiles = {}

        def emit_xload(b, j, split=1):
            x_sb = xp.tile([128, 16, 512], BF, tag="x", name=f"x{b}{j}")
            r0 = (b * 4 + j) * 128
            cper = 8192 // split
            for u in range(split):
                nc.sync.dma_start(
                    x_sb[:, u * (16 // split) : (u + 1) * (16 // split), :],
                    xP[r0 : r0 + 128, u * cper : (u + 1) * cper],
                )
            x_tiles[(b, j)] = x_sb

        # x slice 0 on the Sync DMA queue; weights in parallel on the
        # Scalar engine's queue so the first projection starts ~11us in
        emit_xload(0, 0, split=4)
        for u in range(4):
            nc.scalar.dma_start(
                wq_sb[:, u * 4 : (u + 1) * 4, :],
                wqP[:, u * 4 * DC : (u + 1) * 4 * DC],
            )
        nc.scalar.dma_start(wk_sb[:], wkP[:])
        # wv on the (faster) Sync queue right behind x slice 0
        nc.sync.dma_start(wv_sb[:], wvP[:])
        for h in range(HPC):
            nc.scalar.dma_start(bq_sb[:, h, :], bqd[h * 128 : (h + 1) * 128, :])
            nc.scalar.dma_start(bk_sb[:, h, :], bkd[h * 128 : (h + 1) * 128, :])
        nc.scalar.dma_start(tri_sb[:], trid[:])
        nc.scalar.dma_start(onk_sb[:], onkd[:])

        # per-batch attention tensors (double-buffered across batches)
        qT_tiles, kT_tiles, v_tiles, cN_tiles = {}, {}, {}, {}

        def emit_proj(b, j, bg=None, hook=None):
            if b not in qT_tiles:
                qT_tiles[b] = bp.tile([128, HPC, S], BF, tag="qT", name=f"qT{b}")
                kT_tiles[b] = bp.tile([128, HPC, S], BF, tag="kT", name=f"kT{b}")
                v_tiles[b] = bp.tile([128, 16, DC], BF, tag="v", name=f"v{b}")
            x_sb = x_tiles[(b, j)]
            qT_sb, kT_sb, v_sb = qT_tiles[b], kT_tiles[b], v_tiles[b]
            js = slice(j * 512, (j + 1) * 512)
            first = True
            for w_sb, b_sb, o_sb in (
                (wq_sb, bq_sb, qT_sb),
                (wk_sb, bk_sb, kT_sb),
            ):
                for m in range(HPC):
                    p_ps = ps.tile([128, 512], F32, tag="pr", bufs=2)
                    for t in range(16):
                        nc.tensor.matmul(
                            p_ps[:],
                            w_sb[:, t, m * 128 : (m + 1) * 128],
                            x_sb[:, t, :],
                            start=(t == 0),
                            stop=(t == 15),
                        )
                    nc.scalar.activation(
                        o_sb[:, m, js], p_ps[:], AF.Identity, bias=b_sb[:, m, :]
                    )
                    if first:
                        # boundary work (prev batch's tails + den flush)
                        # lands here so the Q group hides its latency
                        if hook is not None:
                            hook()
                        first = False
                    elif bg is not None:
                        next(bg, None)
            for si in range(4):
                v_ps = ps.tile([128, DC], F32, tag="pr", bufs=2)
                for t in range(16):
                    nc.tensor.matmul(
                        v_ps[:],
                        x_sb[:, t, si * 128 : (si + 1) * 128],
                        wv_sb[:, t, :],
                        start=(t == 0),
                        stop=(t == 15),
                    )
                nc.vector.tensor_copy(v_sb[:, j * 4 + si, :], v_ps[:])
                if bg is not None:
                    next(bg, None)

        # deferred denominator chains + cross-group ctx-matmul tails: both
        # are emitted inside the NEXT head group so the PE never idles on
        # this group's exp/accumulate latency
        pending = []
        tail_q = []

        def flush_pending(n=None):
            cnt = len(pending) if n is None else min(n, len(pending))
            for _ in range(cnt):
                b, qb, h, pacc_bf, ctx_ps = pending.pop(0)
                qs = slice(qb * 512, (qb + 1) * 512)
                den_ps = ps.tile([128, 512], F32, tag="pr", bufs=2)
                nc.tensor.matmul(
                    den_ps[:], onk_sb[:], pacc_bf[:], start=True, stop=True
                )
                lnd_sb = dp.tile([128, 512], F32, tag="lnd", bufs=2)
                nc.scalar.activation(lnd_sb[:], den_ps[:], AF.Ln)
                recb_sb = dp.tile([128, 512], F32, tag="recb", bufs=2)
                nc.scalar.activation(recb_sb[:], lnd_sb[:], AF.Exp, scale=-1.0)
                nc.vector.tensor_tensor(
                    cN_tiles[b][:, h, qs], ctx_ps[:], recb_sb[:], ALU.mult
                )

        def drain_tails(n=None):
            cnt = len(tail_q) if n is None else min(n, len(tail_q))
            for _ in range(cnt):
                tail_q.pop(0)()

        def emit_attn(b, qb, h, bg=None, bg_from=3):
            if b not in cN_tiles:
                cN_tiles[b] = cp.tile([128, HPC, S], BF, tag="cN", name=f"cN{b}")
            qT_sb, kT_sb, v_sb = qT_tiles[b], kT_tiles[b], v_tiles[b]
            q0 = qb * 512
            kmax = 4 * qb + 4
            flush_at = min(5, kmax - 1)
            pts = []
            ctx_ps = ps.tile([128, 512], F32, tag="cx", bufs=2)
            # exp-sum accumulators: even k-chunks on DVE, odd on GpSimd,
            # combined (and cast to bf16) at the end
            pacc_a = dp.tile([128, 512], F32, tag="pacc_a", bufs=2)
            pacc_b = dp.tile([128, 512], F32, tag="pacc_b", bufs=2)
            pacc_bf = dp.tile([128, 512], BF, tag="paccb", bufs=2)
            if qb == 0:
                # odd accumulator's first write only covers [128:512]
                nc.vector.memzero(pacc_b[:, 0:128])

            def ctx_mm(jj):
                p_t, slot, sl = pts[jj]
                nc.tensor.matmul(
                    ctx_ps[:, sl],
                    v_sb[:, jj, h * 128 : (h + 1) * 128],
                    p_t[:, slot, sl],
                    start=(jj == 0),
                    stop=(jj == kmax - 1),
                )

            ctx_next = 0
            for kc in range(0, kmax, 2):
                # previous group's ctx tail while its final exps finish
                drain_tails(2 if kc == 0 else 3)
                if kc <= flush_at <= kc + 1:
                    flush_pending(1)
                # two score chunks share a 2-bank psum tile and ONE exp —
                # halves the per-ACTIVATE fixed overhead on the Scalar engine
                sc_pr = ps.tile([128, 2, 512], F32, tag="sc", bufs=2, name="sc_pr")
                p_t = pp.tile([128, 2, 512], BF, name="p_t")
                sls = []
                for i in range(2):
                    r = kc + i - 4 * qb  # >=0 on the diagonal band
                    sl = slice(r * 128, 512) if r > 0 else slice(0, 512)
                    sls.append((r, sl))
                    nc.tensor.matmul(
                        sc_pr[:, i, sl],
                        kT_sb[:, h, (kc + i) * 128 : (kc + i + 1) * 128],
                        qT_sb[:, h, q0 + sl.start : q0 + 512],
                        start=True,
                        stop=True,
                    )
                # full-pair exp: unused regions hold stale-but-finite scores
                # and are never read downstream
                nc.scalar.activation(p_t[:, :, :], sc_pr[:, :, :], AF.Exp)
                for i in range(2):
                    r, sl = sls[i]
                    if r >= 0:
                        nc.vector.tensor_tensor(
                            p_t[:, i, r * 128 : (r + 1) * 128],
                            p_t[:, i, r * 128 : (r + 1) * 128],
                            tri_sb[:],
                            ALU.mult,
                        )
                    eng, pacc = (
                        (nc.gpsimd, pacc_a) if i == 0 else (nc.vector, pacc_b)
                    )
                    if kc < 2:
                        eng.tensor_copy(pacc[:, sl], p_t[:, i, sl])
                    else:
                        eng.tensor_tensor(
                            pacc[:, sl], pacc[:, sl], p_t[:, i, sl], ALU.add
                        )
                    pts.append((p_t, i, sl))
                while ctx_next <= kc + 1 - LAG:
                    ctx_mm(ctx_next)
                    ctx_next += 1
                if bg is not None:
                    for i in range(2):
                        if kc + i >= bg_from:
                            next(bg, None)

            def tail(jj):
                return lambda: ctx_mm(jj)

            for jj in range(ctx_next, kmax):
                tail_q.append(tail(jj))

            nc.gpsimd.tensor_tensor(pacc_bf[:], pacc_a[:], pacc_b[:], ALU.add)
            pending.append((b, qb, h, pacc_bf, ctx_ps))

        def outproj_gen(b, qb):
            # one yield per (2 matmuls + psum evacuation); interleaved into
            # the next q-block's attention so evac latency hides behind MMs
            cN_sb = cN_tiles[b]
            for qc in range(4):
                r0 = b * S + qb * 512 + qc * 128
                c0 = qb * 512 + qc * 128
                for e2 in range(2):
                    y_sb = yp.tile([128, 1024], BF, tag="y", name="y_sb")
                    for half in range(2):
                        eb = e2 * 1024 + half * 512
                        y_ps = ps.tile([128, 512], F32, tag="pr", bufs=2)
                        nc.tensor.matmul(
                            y_ps[:],
                            cN_sb[:, 0, c0 : c0 + 128],
                            wo_sb[:, 0, eb : eb + 512],
                            start=True,
                            stop=False,
                        )
                        nc.tensor.matmul(
                            y_ps[:],
                            cN_sb[:, 1, c0 : c0 + 128],
                            wo_sb[:, 1, eb : eb + 512],
                            start=False,
                            stop=True,
                        )
                        if half == 0:
                            nc.vector.tensor_copy(y_sb[:, :512], y_ps[:])
                        else:
                            nc.scalar.copy(y_sb[:, 512:], y_ps[:])
                        yield
                    nc.sync.dma_start(
                        yd[r0 : r0 + 128, e2 * 1024 : (e2 + 1) * 1024], y_sb[:]
                    )

        def drain(gen):
            if gen is not None:
                for _ in gen:
                    pass

        carry = None  # outproj generator carried from the previous batch
        for b in range(B):
            def boundary():
                drain_tails()
                flush_pending()

            for j in range(4):
                if b == 0 and j > 0:
                    emit_xload(b, j)
                emit_proj(
                    b,
                    j,
                    bg=carry if j == 0 else None,
                    hook=boundary if j == 0 else None,
                )
                if j == 0:
                    drain(carry)
                    carry = None
            if b == 0:
                # wo is first needed by outproj pulls ~100us in; x prefetch
                # for the next batch ahead of this batch's y writeback
                nc.sync.dma_start(wo_sb[:], woP[:])
                for j in range(4):
                    emit_xload(b + 1, j)
            bg = None
            for qb in range(4):
                # bg units read cN for both heads of qb-1, whose h1 flush
                # happens inside this qb's h0 group at chunk<=5 — pull in
                # h0 only from chunk 6, freely in h1
                emit_attn(b, qb, 0, bg=bg, bg_from=6)
                emit_attn(b, qb, 1, bg=bg, bg_from=3)
                drain(bg)
                bg = outproj_gen(b, qb)
            carry = bg
        drain_tails()
        flush_pending()
        drain(carry)
    _split_excess_waits(nc)
    return nc


def _tri_np():
    kk = np.arange(128)[:, None]
    qq = np.arange(128)[None, :]
    return np.where(kk > qq, 0.0, 1.0).astype(bf16)


def _pack_x(xT):
    # xP[j*128+p, t*512+c] = xT[t*128+p, j*512+c]
    x4 = xT.reshape(16, 128, 8, 512)
    return np.ascontiguousarray(x4.transpose(2, 1, 0, 3).reshape(8 * 128, 8192))


def _pack_w(wT):
    # wP[p, t*C+c] = wT[t*128+p, c]
    t, c = wT.shape[0] // 128, wT.shape[1]
    w3 = wT.reshape(t, 128, c)
    return np.ascontiguousarray(w3.transpose(1, 0, 2).reshape(128, t * c))


def kernel(**inputs):
    global LAST_RESULTS, _NC_CACHE
    x = np.asarray(inputs["x"], np.float32)
    Wq = np.asarray(inputs["Wq"], np.float32)
    bq = np.asarray(inputs["bq"], np.float32)
    Wk = np.asarray(inputs["Wk"], np.float32)
    bk = np.asarray(inputs["bk"], np.float32)
    Wv = np.asarray(inputs["Wv"], np.float32)
    bv = np.asarray(inputs["bv"], np.float32)
    Wo = np.asarray(inputs["Wo"], np.float32)
    bo = np.asarray(inputs["bo"], np.float32)

    xT = np.ascontiguousarray(x.reshape(BS, E).T).astype(bf16)
    xPk = _pack_x(xT)
    tri = _tri_np()
    onesk = np.ones((128, 128), bf16)

    in_maps = []
    for c in range(NCORES):
        dsl = slice(c * DC, (c + 1) * DC)
        in_maps.append(
            {
                "xP": xPk,
                "wqP": _pack_w((Wq[dsl].T * SCALE).astype(bf16)),
                "wkP": _pack_w(Wk[dsl].T.astype(bf16)),
                "wvP": _pack_w(Wv[dsl].T.astype(bf16)),
                "woP": _pack_w(Wo[:, dsl].T.astype(bf16)),
                "bq": np.ascontiguousarray((bq[dsl] * SCALE).reshape(DC, 1)),
                "bk": np.ascontiguousarray(bk[dsl].reshape(DC, 1)),
                "tri": tri,
                "onesk": onesk,
            }
        )

    if _NC_CACHE is None:
        _NC_CACHE = _build()
    res = run_bass_kernel_spmd(_NC_CACHE, in_maps, core_ids=list(range(NCORES)), trace=TRACE)
    LAST_RESULTS = res

    acc = None
    for r in res.results:
        yc = np.asarray(r["y"], np.float32)
        acc = yc if acc is None else acc + yc
    bo_eff = bo + bv @ Wo.T
    acc += bo_eff[None, :]
    return acc.reshape(B, S, E).astype(np.float32)


# revision 22
# speedup vs baseline: 1.1120x; 1.0392x over previous
import sys

sys.path.insert(0, "/opt/trn_rl_repo")

from contextlib import ExitStack

import ml_dtypes
import numpy as np

from concourse import bass, mybir, tile
from concourse.bass_utils import run_bass_kernel_spmd
from concourse.vector_clock import ScopedClock


def _patched_drain_and_barrier(self, tick_clock, wait_clock):
    # Workaround: this compiler rejects a drain carrying >1 sem wait
    # ([NCC_INLA001]); split extra waits onto single-wait nops.
    drain_inst = self.nc.sync.drain()
    wait_clock.add_sem_waits(
        drain_inst.ins, ScopedClock({None: tick_clock.global_clock})
    )
    si = drain_inst.ins.sync_info
    waits = list(si.on_wait) if si and si.on_wait else []
    if len(waits) > 1:
        drain_inst.ins.sync_info = mybir.SyncInfo(
            on_wait=[waits[0]], on_update=list(si.on_update or [])
        )
        for w in waits[1:]:
            nop = self.nc.sync.nop(nofuse=True)
            nop.ins.sync_info = mybir.SyncInfo(on_wait=[w], on_update=[])
    self.nc.all_engine_barrier()
    popped = self.nc._tile_sem_poison_stack.pop()
    assert popped is self._sem_poison
    self.nc.clear_and_free_semaphores(list(self.sems.allocated().values()))
    self.nc.all_engine_barrier()


tile.TileContext._drain_and_barrier = _patched_drain_and_barrier


def _split_excess_waits(nc, limit=1):
    # Workaround: this compiler allows only one sem wait on several
    # instruction encodings; move extra waits onto same-engine nops.
    eng_map = {
        mybir.EngineType.PE: nc.tensor,
        mybir.EngineType.Activation: nc.scalar,
        mybir.EngineType.DVE: nc.vector,
        mybir.EngineType.Pool: nc.gpsimd,
        mybir.EngineType.SP: nc.sync,
    }
    for blk in nc.cur_f.blocks:
        orig = list(blk.instructions)
        out = []
        for ins in orig:
            si = ins.sync_info
            waits = list(si.on_wait) if si and si.on_wait else []
            eng = eng_map.get(ins.engine)
            if len(waits) > limit and eng is not None:
                extra, keep = waits[:-limit], waits[-limit:]
                for w in extra:
                    nop = eng.nop(nofuse=True).ins
                    tail = nc.cur_f.blocks[-1].instructions
                    assert tail[-1] is nop
                    tail.pop()
                    nop.sync_info = mybir.SyncInfo(on_wait=[w], on_update=[])
                    out.append(nop)
                ins.sync_info = mybir.SyncInfo(
                    on_wait=keep, on_update=list(si.on_update or [])
                )
            out.append(ins)
        blk.instructions[:] = out

bf16 = ml_dtypes.bfloat16
BF = bass.mybir.dt.bfloat16
F32 = bass.mybir.dt.float32
AF = mybir.ActivationFunctionType
ALU = mybir.AluOpType

B, S, E, H, D = 2, 2048, 2048, 16, 128
BS = B * S
NCORES = 8
HPC = H // NCORES  # heads per core
DC = HPC * D  # per-core head-dim width (256)
SCALE = 1.0 / float(np.sqrt(D))
LAG = 4  # ctx matmul lags sc matmul by this many k-chunks

TRACE = False
LAST_RESULTS = None
_NC_CACHE = None


def _build():
    nc = bass.Bass()
    # xP: slice-major packed x — row j*128+p holds x[t*128+p, j*512+c] at
    # col t*512+c, so one DMA per 512-token slice reads 16KB-contiguous rows
    xP = nc.declare_dram_parameter("xP", (8 * 128, 16 * 512), BF, isOutput=False)
    # weights packed so SBUF partition p's row is contiguous in DRAM
    wqP = nc.declare_dram_parameter("wqP", (128, 16 * DC), BF, isOutput=False)
    wkP = nc.declare_dram_parameter("wkP", (128, 16 * DC), BF, isOutput=False)
    wvP = nc.declare_dram_parameter("wvP", (128, 16 * DC), BF, isOutput=False)
    woP = nc.declare_dram_parameter("woP", (128, HPC * E), BF, isOutput=False)
    bqd = nc.declare_dram_parameter("bq", (DC, 1), F32, isOutput=False)
    bkd = nc.declare_dram_parameter("bk", (DC, 1), F32, isOutput=False)
    trid = nc.declare_dram_parameter("tri", (128, 128), BF, isOutput=False)
    onkd = nc.declare_dram_parameter("onesk", (128, 128), BF, isOutput=False)
    yd = nc.declare_dram_parameter("y", (BS, E), BF, isOutput=True)

    with ExitStack() as ctx:
        tc = ctx.enter_context(tile.TileContext(nc))
        wp = ctx.enter_context(tc.tile_pool(name="wp", bufs=1))
        xp = ctx.enter_context(tc.tile_pool(name="xp", bufs=4))
        bp = ctx.enter_context(tc.tile_pool(name="bp", bufs=2))
        cp = ctx.enter_context(tc.tile_pool(name="cp", bufs=1))
        pp = ctx.enter_context(tc.tile_pool(name="pp", bufs=6))
        dp = ctx.enter_context(tc.tile_pool(name="dp", bufs=2))
        yp = ctx.enter_context(tc.tile_pool(name="yp", bufs=3))
        ps = ctx.enter_context(tc.tile_pool(name="ps", bufs=1, space="PSUM"))

        wq_sb = wp.tile([128, 16, DC], BF)
        wk_sb = wp.tile([128, 16, DC], BF)
        wv_sb = wp.tile([128, 16, DC], BF)
        wo_sb = wp.tile([128, HPC, E], BF)
        bq_sb = wp.tile([128, HPC, 1], F32)
        bk_sb = wp.tile([128, HPC, 1], F32)
        tri_sb = wp.tile([128, 128], BF)
        onk_sb = wp.tile([128, 128], BF)

        x_tiles = {}

        def emit_xload(b, j, split=1):
            x_sb = xp.tile([128, 16, 512], BF, tag="x", name=f"x{b}{j}")
            r0 = (b * 4 + j) * 128
            cper = 8192 // split
            for u in range(split):
                nc.sync.dma_start(
                    x_sb[:, u * (16 // split) : (u + 1) * (16 // split), :],
                    xP[r0 : r0 + 128, u * cper : (u + 1) * cper],
                )
            x_tiles[(b, j)] = x_sb

        # x slice 0 on the Sync DMA queue; weights in parallel on the
        # Scalar engine's queue so the first projection starts ~11us in
        emit_xload(0, 0, split=4)
        for u in range(4):
            nc.scalar.dma_start(
                wq_sb[:, u * 4 : (u + 1) * 4, :],
                wqP[:, u * 4 * DC : (u + 1) * 4 * DC],
            )
        nc.scalar.dma_start(wk_sb[:], wkP[:])
        # wv on the (faster) Sync queue right behind x slice 0
        nc.sync.dma_start(wv_sb[:], wvP[:])
        for h in range(HPC):
            nc.scalar.dma_start(bq_sb[:, h, :], bqd[h * 128 : (h + 1) * 128, :])
            nc.scalar.dma_start(bk_sb[:, h, :], bkd[h * 128 : (h + 1) * 128, :])
        nc.scalar.dma_start(tri_sb[:], trid[:])
        nc.scalar.dma_start(onk_sb[:], onkd[:])

        # per-batch attention tensors (double-buffered across batches)
        qT_tiles, kT_tiles, v_tiles, cN_tiles = {}, {}, {}, {}

        def emit_proj(b, j, bg=None, hook=None):
            if b not in qT_tiles:
                qT_tiles[b] = bp.tile([128, HPC, S], BF, tag="qT", name=f"qT{b}")
                kT_tiles[b] = bp.tile([128, HPC, S], BF, tag="kT", name=f"kT{b}")
                v_tiles[b] = bp.tile([128, 16, DC], BF, tag="v", name=f"v{b}")
            x_sb = x_tiles[(b, j)]
            qT_sb, kT_sb, v_sb = qT_tiles[b], kT_tiles[b], v_tiles[b]
            js = slice(j * 512, (j + 1) * 512)
            first = True
            for w_sb, b_sb, o_sb in (
                (wq_sb, bq_sb, qT_sb),
                (wk_sb, bk_sb, kT_sb),
            ):
                for m in range(HPC):
                    p_ps = ps.tile([128, 512], F32, tag="pr", bufs=2)
                    for t in range(16):
                        nc.tensor.matmul(
                            p_ps[:],
                            w_sb[:, t, m * 128 : (m + 1) * 128],
                            x_sb[:, t, :],
                            start=(t == 0),
                            stop=(t == 15),
                        )
                    nc.scalar.activation(
                        o_sb[:, m, js], p_ps[:], AF.Identity, bias=b_sb[:, m, :]
                    )
                    if first:
                        # boundary work (prev batch's tails + den flush)
                        # lands here so the Q group hides its latency
                        if hook is not None:
                            hook()
                        first = False
                    elif bg is not None:
                        next(bg, None)
            for si in range(4):
                v_ps = ps.tile([128, DC], F32, tag="pr", bufs=2)
                for t in range(16):
                    nc.tensor.matmul(
                        v_ps[:],
                        x_sb[:, t, si * 128 : (si + 1) * 128],
                        wv_sb[:, t, :],
                        start=(t == 0),
                        stop=(t == 15),
                    )
                nc.vector.tensor_copy(v_sb[:, j * 4 + si, :], v_ps[:])
                if bg is not None:
                    next(bg, None)

        # deferred denominator chains + cross-group ctx-matmul tails: both
        # are emitted inside the NEXT head group so the PE never idles on
        # this group's exp/accumulate latency
        pending = []
        tail_q = []

        def flush_pending(n=None):
            cnt = len(pending) if n is None else min(n, len(pending))
            for _ in range(cnt):
                b, qb, h, pacc_bf, ctx_ps = pending.pop(0)
                qs = slice(qb * 512, (qb + 1) * 512)
                den_ps = ps.tile([128, 512], F32, tag="pr", bufs=2)
                nc.tensor.matmul(
                    den_ps[:], onk_sb[:], pacc_bf[:], start=True, stop=True
                )
                lnd_sb = dp.tile([128, 512], F32, tag="lnd", bufs=2)
                nc.scalar.activation(lnd_sb[:], den_ps[:], AF.Ln)
                recb_sb = dp.tile([128, 512], F32, tag="recb", bufs=2)
                nc.scalar.activation(recb_sb[:], lnd_sb[:], AF.Exp, scale=-1.0)
                nc.vector.tensor_tensor(
                    cN_tiles[b][:, h, qs], ctx_ps[:], recb_sb[:], ALU.mult
                )

        def drain_tails(n=None):
            cnt = len(tail_q) if n is None else min(n, len(tail_q))
            for _ in range(cnt):
                tail_q.pop(0)()

        def emit_attn(b, qb, h, bg=None, bg_from=3):
            if b not in cN_tiles:
                cN_tiles[b] = cp.tile([128, HPC, S], BF, tag="cN", name=f"cN{b}")
            qT_sb, kT_sb, v_sb = qT_tiles[b], kT_tiles[b], v_tiles[b]
            q0 = qb * 512
            kmax = 4 * qb + 4
            flush_at = min(5, kmax - 1)
            pts = []
            ctx_ps = ps.tile([128, 512], F32, tag="cx", bufs=2)
            # exp-sum accumulators: even k-chunks on DVE, odd on GpSimd,
            # combined (and cast to bf16) at the end
            pacc_a = dp.tile([128, 512], F32, tag="pacc_a", bufs=2)
            pacc_b = dp.tile([128, 512], F32, tag="pacc_b", bufs=2)
            pacc_bf = dp.tile([128, 512], BF, tag="paccb", bufs=2)
            if qb == 0:
                # odd accumulator's first write only covers [128:512]
                nc.gpsimd.memzero(pacc_b[:, 0:128])

            def ctx_mm(jj):
                p_t, slot, sl = pts[jj]
                nc.tensor.matmul(
                    ctx_ps[:, sl],
                    v_sb[:, jj, h * 128 : (h + 1) * 128],
                    p_t[:, slot, sl],
                    start=(jj == 0),
                    stop=(jj == kmax - 1),
                )

            ctx_next = 0
            for kc in range(0, kmax, 2):
                # previous group's ctx tail while its final exps finish
                drain_tails(2 if kc == 0 else 3)
                if kc <= flush_at <= kc + 1:
                    flush_pending(1)
                # two score chunks share a 2-bank psum tile and ONE exp —
                # halves the per-ACTIVATE fixed overhead on the Scalar engine
                sc_pr = ps.tile([128, 2, 512], F32, tag="sc", bufs=2, name="sc_pr")
                p_t = pp.tile([128, 2, 512], BF, name="p_t")
                sls = []
                for i in range(2):
                    r = kc + i - 4 * qb  # >=0 on the diagonal band
                    sl = slice(r * 128, 512) if r > 0 else slice(0, 512)
                    sls.append((r, sl))
                    nc.tensor.matmul(
                        sc_pr[:, i, sl],
                        kT_sb[:, h, (kc + i) * 128 : (kc + i + 1) * 128],
                        qT_sb[:, h, q0 + sl.start : q0 + 512],
                        start=True,
                        stop=True,
                    )
                # full-pair exp: unused regions hold stale-but-finite scores
                # and are never read downstream
                nc.scalar.activation(p_t[:, :, :], sc_pr[:, :, :], AF.Exp)
                for i in range(2):
                    r, sl = sls[i]
                    if r >= 0:
                        nc.vector.tensor_tensor(
                            p_t[:, i, r * 128 : (r + 1) * 128],
                            p_t[:, i, r * 128 : (r + 1) * 128],
                            tri_sb[:],
                            ALU.mult,
                        )
                    eng, pacc = (
                        (nc.vector, pacc_a) if i == 0 else (nc.gpsimd, pacc_b)
                    )
                    if kc < 2:
                        eng.tensor_copy(pacc[:, sl], p_t[:, i, sl])
                    else:
                        eng.tensor_tensor(
                            pacc[:, sl], pacc[:, sl], p_t[:, i, sl], ALU.add
                        )
                    pts.append((p_t, i, sl))
                while ctx_next <= kc + 1 - LAG:
                    ctx_mm(ctx_next)
                    ctx_next += 1
                if bg is not None:
                    for i in range(2):
                        if kc + i >= bg_from:
                            next(bg, None)

            def tail(jj):
                return lambda: ctx_mm(jj)

            for jj in range(ctx_next, kmax):
                tail_q.append(tail(jj))

            def fin():
                nc.gpsimd.tensor_tensor(pacc_bf[:], pacc_a[:], pacc_b[:], ALU.add)
                pending.append((b, qb, h, pacc_bf, ctx_ps))

            tail_q.append(fin)

        def outproj_gen(b, qb):
            # one yield per (2 matmuls + psum evacuation); interleaved into
            # the next q-block's attention so evac latency hides behind MMs
            cN_sb = cN_tiles[b]
            for qc in range(4):
                r0 = b * S + qb * 512 + qc * 128
                c0 = qb * 512 + qc * 128
                for e2 in range(2):
                    y_sb = yp.tile([128, 1024], BF, tag="y", name="y_sb")
                    for half in range(2):
                        eb = e2 * 1024 + half * 512
                        y_ps = ps.tile([128, 512], F32, tag="pr", bufs=2)
                        nc.tensor.matmul(
                            y_ps[:],
                            cN_sb[:, 0, c0 : c0 + 128],
                            wo_sb[:, 0, eb : eb + 512],
                            start=True,
                            stop=False,
                        )
                        nc.tensor.matmul(
                            y_ps[:],
                            cN_sb[:, 1, c0 : c0 + 128],
                            wo_sb[:, 1, eb : eb + 512],
                            start=False,
                            stop=True,
                        )
                        if half == 0:
                            nc.vector.tensor_copy(y_sb[:, :512], y_ps[:])
                        else:
                            nc.scalar.copy(y_sb[:, 512:], y_ps[:])
                        yield
                    nc.sync.dma_start(
                        yd[r0 : r0 + 128, e2 * 1024 : (e2 + 1) * 1024], y_sb[:]
                    )

        def drain(gen):
            if gen is not None:
                for _ in gen:
                    pass

        carry = None  # outproj generator carried from the previous batch
        for b in range(B):
            def boundary():
                drain_tails()
                flush_pending()

            for j in range(4):
                if b == 0 and j > 0:
                    emit_xload(b, j)
                emit_proj(
                    b,
                    j,
                    bg=carry if j == 0 else None,
                    hook=boundary if j == 0 else None,
                )
                if j == 0:
                    drain(carry)
                    carry = None
            if b == 0:
                # wo is first needed by outproj pulls ~100us in; x prefetch
                # for the next batch ahead of this batch's y writeback
                nc.sync.dma_start(wo_sb[:], woP[:])
                for j in range(4):
                    emit_xload(b + 1, j)
            bg = None
            for qb in range(4):
                # bg units read cN for both heads of qb-1, whose h1 flush
                # happens inside this qb's h0 group at chunk<=5 — pull in
                # h0 only from chunk 6, freely in h1
                emit_attn(b, qb, 0, bg=bg, bg_from=6)
                emit_attn(b, qb, 1, bg=bg, bg_from=3)
                drain(bg)
                bg = outproj_gen(b, qb)
            carry = bg
        drain_tails()
        flush_pending()
        drain(carry)
    _split_excess_waits(nc)
    return nc


def _tri_np():
    kk = np.arange(128)[:, None]
    qq = np.arange(128)[None, :]
    return np.where(kk > qq, 0.0, 1.0).astype(bf16)


def _pack_x(xT):
    # xP[j*128+p, t*512+c] = xT[t*128+p, j*512+c]
    x4 = xT.reshape(16, 128, 8, 512)
    return np.ascontiguousarray(x4.transpose(2, 1, 0, 3).reshape(8 * 128, 8192))


def _pack_w(wT):
    # wP[p, t*C+c] = wT[t*128+p, c]
    t, c = wT.shape[0] // 128, wT.shape[1]
    w3 = wT.reshape(t, 128, c)
    return np.ascontiguousarray(w3.transpose(1, 0, 2).reshape(128, t * c))


def kernel(**inputs):
    global LAST_RESULTS, _NC_CACHE
    x = np.asarray(inputs["x"], np.float32)
    Wq = np.asarray(inputs["Wq"], np.float32)
    bq = np.asarray(inputs["bq"], np.float32)
    Wk = np.asarray(inputs["Wk"], np.float32)
    bk = np.asarray(inputs["bk"], np.float32)
    Wv = np.asarray(inputs["Wv"], np.float32)
    bv = np.asarray(inputs["bv"], np.float32)
    Wo = np.asarray(inputs["Wo"], np.float32)
    bo = np.asarray(inputs["bo"], np.float32)

    xT = np.ascontiguousarray(x.reshape(BS, E).T).astype(bf16)
    xPk = _pack_x(xT)
    tri = _tri_np()
    onesk = np.ones((128, 128), bf16)

    in_maps = []
    for c in range(NCORES):
        dsl = slice(c * DC, (c + 1) * DC)
        in_maps.append(
            {
                "xP": xPk,
                "wqP": _pack_w((Wq[dsl].T * SCALE).astype(bf16)),
                "wkP": _pack_w(Wk[dsl].T.astype(bf16)),
                "wvP": _pack_w(Wv[dsl].T.astype(bf16)),
                "woP": _pack_w(Wo[:, dsl].T.astype(bf16)),
                "bq": np.ascontiguousarray((bq[dsl] * SCALE).reshape(DC, 1)),
                "bk": np.ascontiguousarray(bk[dsl].reshape(DC, 1)),
                "tri": tri,
                "onesk": onesk,
            }
        )

    if _NC_CACHE is None:
        _NC_CACHE = _build()
    res = run_bass_kernel_spmd(_NC_CACHE, in_maps, core_ids=list(range(NCORES)), trace=TRACE)
    LAST_RESULTS = res

    acc = None
    for r in res.results:
        yc = np.asarray(r["y"], np.float32)
        acc = yc if acc is None else acc + yc
    bo_eff = bo + bv @ Wo.T
    acc += bo_eff[None, :]
    return acc.reshape(B, S, E).astype(np.float32)


# revision 24
# speedup vs baseline: 1.1318x; 1.0178x over previous
import sys

sys.path.insert(0, "/opt/trn_rl_repo")

from contextlib import ExitStack

import ml_dtypes
import numpy as np

from concourse import bass, mybir, tile
from concourse.bass_utils import run_bass_kernel_spmd
from concourse.vector_clock import ScopedClock


def _patched_drain_and_barrier(self, tick_clock, wait_clock):
    # Workaround: this compiler rejects a drain carrying >1 sem wait
    # ([NCC_INLA001]); split extra waits onto single-wait nops.
    drain_inst = self.nc.sync.drain()
    wait_clock.add_sem_waits(
        drain_inst.ins, ScopedClock({None: tick_clock.global_clock})
    )
    si = drain_inst.ins.sync_info
    waits = list(si.on_wait) if si and si.on_wait else []
    if len(waits) > 1:
        drain_inst.ins.sync_info = mybir.SyncInfo(
            on_wait=[waits[0]], on_update=list(si.on_update or [])
        )
        for w in waits[1:]:
            nop = self.nc.sync.nop(nofuse=True)
            nop.ins.sync_info = mybir.SyncInfo(on_wait=[w], on_update=[])
    self.nc.all_engine_barrier()
    popped = self.nc._tile_sem_poison_stack.pop()
    assert popped is self._sem_poison
    self.nc.clear_and_free_semaphores(list(self.sems.allocated().values()))
    self.nc.all_engine_barrier()


tile.TileContext._drain_and_barrier = _patched_drain_and_barrier


def _split_excess_waits(nc, limit=1):
    # Workaround: this compiler allows only one sem wait on several
    # instruction encodings; move extra waits onto same-engine nops.
    eng_map = {
        mybir.EngineType.PE: nc.tensor,
        mybir.EngineType.Activation: nc.scalar,
        mybir.EngineType.DVE: nc.vector,
        mybir.EngineType.Pool: nc.gpsimd,
        mybir.EngineType.SP: nc.sync,
    }
    for blk in nc.cur_f.blocks:
        orig = list(blk.instructions)
        out = []
        for ins in orig:
            si = ins.sync_info
            waits = list(si.on_wait) if si and si.on_wait else []
            eng = eng_map.get(ins.engine)
            if len(waits) > limit and eng is not None:
                extra, keep = waits[:-limit], waits[-limit:]
                for w in extra:
                    nop = eng.nop(nofuse=True).ins
                    tail = nc.cur_f.blocks[-1].instructions
                    assert tail[-1] is nop
                    tail.pop()
                    nop.sync_info = mybir.SyncInfo(on_wait=[w], on_update=[])
                    out.append(nop)
                ins.sync_info = mybir.SyncInfo(
                    on_wait=keep, on_update=list(si.on_update or [])
                )
            out.append(ins)
        blk.instructions[:] = out

bf16 = ml_dtypes.bfloat16
BF = bass.mybir.dt.bfloat16
F32 = bass.mybir.dt.float32
F32R = bass.mybir.dt.float32r
AF = mybir.ActivationFunctionType
ALU = mybir.AluOpType

B, S, E, H, D = 2, 2048, 2048, 16, 128
BS = B * S
NCORES = 8
HPC = H // NCORES  # heads per core
DC = HPC * D  # per-core head-dim width (256)
SCALE = 1.0 / float(np.sqrt(D))
LAG = 4  # ctx matmul lags sc matmul by this many k-chunks

TRACE = False
LAST_RESULTS = None
_NC_CACHE = None


def _build():
    nc = bass.Bass()
    # xP: slice-major packed x — row j*128+p holds x[t*128+p, j*512+c] at
    # col t*512+c, so one DMA per 512-token slice reads 16KB-contiguous rows
    xP = nc.declare_dram_parameter("xP", (8 * 128, 16 * 512), BF, isOutput=False)
    # weights packed so SBUF partition p's row is contiguous in DRAM
    wqP = nc.declare_dram_parameter("wqP", (128, 16 * DC), BF, isOutput=False)
    wkP = nc.declare_dram_parameter("wkP", (128, 16 * DC), BF, isOutput=False)
    wvP = nc.declare_dram_parameter("wvP", (128, 16 * DC), BF, isOutput=False)
    woP = nc.declare_dram_parameter("woP", (128, HPC * E), BF, isOutput=False)
    bqd = nc.declare_dram_parameter("bq", (DC, 1), F32, isOutput=False)
    bkd = nc.declare_dram_parameter("bk", (DC, 1), F32, isOutput=False)
    trid = nc.declare_dram_parameter("tri", (128, 128), BF, isOutput=False)
    onkd = nc.declare_dram_parameter("onesk", (128, 128), F32, isOutput=False)
    yd = nc.declare_dram_parameter("y", (BS, E), BF, isOutput=True)

    with ExitStack() as ctx:
        tc = ctx.enter_context(tile.TileContext(nc))
        wp = ctx.enter_context(tc.tile_pool(name="wp", bufs=1))
        xp = ctx.enter_context(tc.tile_pool(name="xp", bufs=4))
        bp = ctx.enter_context(tc.tile_pool(name="bp", bufs=2))
        cp = ctx.enter_context(tc.tile_pool(name="cp", bufs=1))
        pp = ctx.enter_context(tc.tile_pool(name="pp", bufs=6))
        dp = ctx.enter_context(tc.tile_pool(name="dp", bufs=2))
        yp = ctx.enter_context(tc.tile_pool(name="yp", bufs=3))
        ps = ctx.enter_context(tc.tile_pool(name="ps", bufs=1, space="PSUM"))

        wq_sb = wp.tile([128, 16, DC], BF)
        wk_sb = wp.tile([128, 16, DC], BF)
        wv_sb = wp.tile([128, 16, DC], BF)
        wo_sb = wp.tile([128, HPC, E], BF)
        bq_sb = wp.tile([128, HPC, 1], F32)
        bk_sb = wp.tile([128, HPC, 1], F32)
        tri_sb = wp.tile([128, 128], BF)
        onk_sb = wp.tile([128, 128], F32)
        onk_r = wp.tile([128, 128], mybir.dt.float32r)
        wrm_sb = wp.tile([128, 128], BF)

        x_tiles = {}

        def emit_xload(b, j, split=1):
            x_sb = xp.tile([128, 16, 512], BF, tag="x", name=f"x{b}{j}")
            r0 = (b * 4 + j) * 128
            cper = 8192 // split
            for u in range(split):
                nc.sync.dma_start(
                    x_sb[:, u * (16 // split) : (u + 1) * (16 // split), :],
                    xP[r0 : r0 + 128, u * cper : (u + 1) * cper],
                )
            x_tiles[(b, j)] = x_sb

        # warm the PE (HAM un-throttle needs ~3.4us of activity) while the
        # first DMAs are in flight; the zeroed tile needs no DMA
        nc.vector.memzero(wrm_sb[:])
        wrm_ps = ps.tile([128, 64], F32, tag="pr", bufs=2, name="wrm_ps")
        for _ in range(55):
            nc.tensor.matmul(wrm_ps[:], wrm_sb[:], wrm_sb[:, 0:64], start=True, stop=True)

        # x slice 0 on the Sync DMA queue; weights in parallel on the
        # Scalar engine's queue so the first projection starts ~11us in
        emit_xload(0, 0, split=4)
        for u in range(4):
            nc.scalar.dma_start(
                wq_sb[:, u * 4 : (u + 1) * 4, :],
                wqP[:, u * 4 * DC : (u + 1) * 4 * DC],
            )
        nc.scalar.dma_start(wk_sb[:], wkP[:])
        # wv on the (faster) Sync queue right behind x slice 0
        nc.sync.dma_start(wv_sb[:], wvP[:])
        for h in range(HPC):
            nc.scalar.dma_start(bq_sb[:, h, :], bqd[h * 128 : (h + 1) * 128, :])
            nc.scalar.dma_start(bk_sb[:, h, :], bkd[h * 128 : (h + 1) * 128, :])
        nc.scalar.dma_start(tri_sb[:], trid[:])
        nc.scalar.dma_start(onk_sb[:], onkd[:])
        nc.vector.tensor_copy(onk_r[:], onk_sb[:])

        # per-batch attention tensors (double-buffered across batches)
        qT_tiles, kT_tiles, v_tiles, cN_tiles = {}, {}, {}, {}

        def emit_proj(b, j, bg=None, hook=None):
            if b not in qT_tiles:
                qT_tiles[b] = bp.tile([128, HPC, S], BF, tag="qT", name=f"qT{b}")
                kT_tiles[b] = bp.tile([128, HPC, S], BF, tag="kT", name=f"kT{b}")
                v_tiles[b] = bp.tile([128, 16, DC], BF, tag="v", name=f"v{b}")
            x_sb = x_tiles[(b, j)]
            qT_sb, kT_sb, v_sb = qT_tiles[b], kT_tiles[b], v_tiles[b]
            js = slice(j * 512, (j + 1) * 512)
            first = True
            for w_sb, b_sb, o_sb in (
                (wq_sb, bq_sb, qT_sb),
                (wk_sb, bk_sb, kT_sb),
            ):
                for m in range(HPC):
                    p_ps = ps.tile([128, 512], F32, tag="pr", bufs=2)
                    for t in range(16):
                        nc.tensor.matmul(
                            p_ps[:],
                            w_sb[:, t, m * 128 : (m + 1) * 128],
                            x_sb[:, t, :],
                            start=(t == 0),
                            stop=(t == 15),
                        )
                    nc.scalar.activation(
                        o_sb[:, m, js], p_ps[:], AF.Identity, bias=b_sb[:, m, :]
                    )
                    if first:
                        # boundary work (prev batch's tails + den flush)
                        # lands here so the Q group hides its latency
                        if hook is not None:
                            hook()
                        first = False
                    elif bg is not None:
                        next(bg, None)
            for si in range(4):
                v_ps = ps.tile([128, DC], F32, tag="pr", bufs=2)
                for t in range(16):
                    nc.tensor.matmul(
                        v_ps[:],
                        x_sb[:, t, si * 128 : (si + 1) * 128],
                        wv_sb[:, t, :],
                        start=(t == 0),
                        stop=(t == 15),
                    )
                nc.vector.tensor_copy(v_sb[:, j * 4 + si, :], v_ps[:])
                if bg is not None:
                    next(bg, None)

        # deferred denominator chains + cross-group ctx-matmul tails: both
        # are emitted inside the NEXT head group so the PE never idles on
        # this group's exp/accumulate latency
        pending = []
        tail_q = []

        def flush_pending(n=None):
            cnt = len(pending) if n is None else min(n, len(pending))
            for _ in range(cnt):
                b, qb, h, pacc_a, pacc_b, ctx_ps = pending.pop(0)
                qs = slice(qb * 512, (qb + 1) * 512)
                den_ps = ps.tile([128, 512], F32, tag="pr", bufs=2)
                nc.tensor.matmul(
                    den_ps[:], onk_r[:], pacc_a[:], start=True, stop=False
                )
                nc.tensor.matmul(
                    den_ps[:], onk_r[:], pacc_b[:], start=False, stop=True
                )
                lnd_sb = dp.tile([128, 512], F32, tag="lnd", bufs=2)
                nc.scalar.activation(lnd_sb[:], den_ps[:], AF.Ln)
                recb_sb = dp.tile([128, 512], F32, tag="recb", bufs=2)
                nc.scalar.activation(recb_sb[:], lnd_sb[:], AF.Exp, scale=-1.0)
                nc.vector.tensor_tensor(
                    cN_tiles[b][:, h, qs], ctx_ps[:], recb_sb[:], ALU.mult
                )

        def drain_tails(n=None):
            cnt = len(tail_q) if n is None else min(n, len(tail_q))
            for _ in range(cnt):
                tail_q.pop(0)()

        def emit_attn(b, qb, h, bg=None, bg_from=3):
            if b not in cN_tiles:
                cN_tiles[b] = cp.tile([128, HPC, S], BF, tag="cN", name=f"cN{b}")
            qT_sb, kT_sb, v_sb = qT_tiles[b], kT_tiles[b], v_tiles[b]
            q0 = qb * 512
            kmax = 4 * qb + 4
            flush_at = min(5, kmax - 1)
            pts = []
            ctx_ps = ps.tile([128, 512], F32, tag="cx", bufs=2)
            # exp-sum accumulators: even k-chunks on DVE, odd on GpSimd,
            # combined (and cast to bf16) at the end
            pacc_a = dp.tile([128, 512], F32R, tag="pacc_a", bufs=2)
            pacc_b = dp.tile([128, 512], F32R, tag="pacc_b", bufs=2)
            if qb == 0:
                # odd accumulator's first write only covers [128:512]
                nc.gpsimd.memzero(pacc_b[:, 0:128])

            def ctx_mm(jj):
                p_t, slot, sl = pts[jj]
                nc.tensor.matmul(
                    ctx_ps[:, sl],
                    v_sb[:, jj, h * 128 : (h + 1) * 128],
                    p_t[:, slot, sl],
                    start=(jj == 0),
                    stop=(jj == kmax - 1),
                )

            ctx_next = 0
            for kc in range(0, kmax, 2):
                # previous group's ctx tail while its final exps finish
                drain_tails(2 if kc == 0 else 3)
                if kc <= flush_at <= kc + 1:
                    flush_pending(1)
                # two score chunks share a 2-bank psum tile and ONE exp —
                # halves the per-ACTIVATE fixed overhead on the Scalar engine
                sc_pr = ps.tile([128, 2, 512], F32, tag="sc", bufs=2, name="sc_pr")
                p_t = pp.tile([128, 2, 512], BF, name="p_t")
                sls = []
                for i in range(2):
                    r = kc + i - 4 * qb  # >=0 on the diagonal band
                    sl = slice(r * 128, 512) if r > 0 else slice(0, 512)
                    sls.append((r, sl))
                    nc.tensor.matmul(
                        sc_pr[:, i, sl],
                        kT_sb[:, h, (kc + i) * 128 : (kc + i + 1) * 128],
                        qT_sb[:, h, q0 + sl.start : q0 + 512],
                        start=True,
                        stop=True,
                    )
                # full-pair exp: unused regions hold stale-but-finite scores
                # and are never read downstream
                nc.scalar.activation(p_t[:, :, :], sc_pr[:, :, :], AF.Exp)
                for i in range(2):
                    r, sl = sls[i]
                    if r >= 0:
                        nc.vector.tensor_tensor(
                            p_t[:, i, r * 128 : (r + 1) * 128],
                            p_t[:, i, r * 128 : (r + 1) * 128],
                            tri_sb[:],
                            ALU.mult,
                        )
                    eng, pacc = (
                        (nc.vector, pacc_a) if i == 0 else (nc.gpsimd, pacc_b)
                    )
                    if kc < 2:
                        eng.tensor_copy(pacc[:, sl], p_t[:, i, sl])
                    else:
                        eng.tensor_tensor(
                            pacc[:, sl], pacc[:, sl], p_t[:, i, sl], ALU.add
                        )
                    pts.append((p_t, i, sl))
                while ctx_next <= kc + 1 - LAG:
                    ctx_mm(ctx_next)
                    ctx_next += 1
                if bg is not None:
                    for i in range(2):
                        if kc + i >= bg_from:
                            next(bg, None)

            def tail(jj):
                return lambda: ctx_mm(jj)

            for jj in range(ctx_next, kmax):
                tail_q.append(tail(jj))

            pending.append((b, qb, h, pacc_a, pacc_b, ctx_ps))

        def outproj_gen(b, qb):
            # one yield per (2 matmuls + psum evacuation); interleaved into
            # the next q-block's attention so evac latency hides behind MMs
            cN_sb = cN_tiles[b]
            for qc in range(4):
                r0 = b * S + qb * 512 + qc * 128
                c0 = qb * 512 + qc * 128
                for e2 in range(2):
                    y_sb = yp.tile([128, 1024], BF, tag="y", name="y_sb")
                    for half in range(2):
                        eb = e2 * 1024 + half * 512
                        y_ps = ps.tile([128, 512], F32, tag="pr", bufs=2)
                        nc.tensor.matmul(
                            y_ps[:],
                            cN_sb[:, 0, c0 : c0 + 128],
                            wo_sb[:, 0, eb : eb + 512],
                            start=True,
                            stop=False,
                        )
                        nc.tensor.matmul(
                            y_ps[:],
                            cN_sb[:, 1, c0 : c0 + 128],
                            wo_sb[:, 1, eb : eb + 512],
                            start=False,
                            stop=True,
                        )
                        if half == 0:
                            nc.vector.tensor_copy(y_sb[:, :512], y_ps[:])
                        else:
                            nc.scalar.copy(y_sb[:, 512:], y_ps[:])
                        yield
                    nc.sync.dma_start(
                        yd[r0 : r0 + 128, e2 * 1024 : (e2 + 1) * 1024], y_sb[:]
                    )

        def drain(gen):
            if gen is not None:
                for _ in gen:
                    pass

        carry = None  # outproj generator carried from the previous batch
        for b in range(B):
            def boundary():
                drain_tails()
                flush_pending()

            for j in range(4):
                if b == 0 and j > 0:
                    emit_xload(b, j)
                emit_proj(
                    b,
                    j,
                    bg=carry if j == 0 else None,
                    hook=boundary if j == 0 else None,
                )
                if j == 0:
                    drain(carry)
                    carry = None
            if b == 0:
                # wo is first needed by outproj pulls ~100us in; x prefetch
                # for the next batch ahead of this batch's y writeback
                nc.sync.dma_start(wo_sb[:], woP[:])
                for j in range(4):
                    emit_xload(b + 1, j)
            bg = None
            for qb in range(4):
                # bg units read cN for both heads of qb-1, whose h1 flush
                # happens inside this qb's h0 group at chunk<=5 — pull in
                # h0 only from chunk 6, freely in h1
                emit_attn(b, qb, 0, bg=bg, bg_from=6)
                emit_attn(b, qb, 1, bg=bg, bg_from=3)
                drain(bg)
                bg = outproj_gen(b, qb)
            carry = bg
        drain_tails()
        flush_pending()
        drain(carry)
    _split_excess_waits(nc)
    return nc


def _tri_np():
    kk = np.arange(128)[:, None]
    qq = np.arange(128)[None, :]
    return np.where(kk > qq, 0.0, 1.0).astype(bf16)


def _pack_x(xT):
    # xP[j*128+p, t*512+c] = xT[t*128+p, j*512+c]
    x4 = xT.reshape(16, 128, 8, 512)
    return np.ascontiguousarray(x4.transpose(2, 1, 0, 3).reshape(8 * 128, 8192))


def _pack_w(wT):
    # wP[p, t*C+c] = wT[t*128+p, c]
    t, c = wT.shape[0] // 128, wT.shape[1]
    w3 = wT.reshape(t, 128, c)
    return np.ascontiguousarray(w3.transpose(1, 0, 2).reshape(128, t * c))


def kernel(**inputs):
    global LAST_RESULTS, _NC_CACHE
    x = np.asarray(inputs["x"], np.float32)
    Wq = np.asarray(inputs["Wq"], np.float32)
    bq = np.asarray(inputs["bq"], np.float32)
    Wk = np.asarray(inputs["Wk"], np.float32)
    bk = np.asarray(inputs["bk"], np.float32)
    Wv = np.asarray(inputs["Wv"], np.float32)
    bv = np.asarray(inputs["bv"], np.float32)
    Wo = np.asarray(inputs["Wo"], np.float32)
    bo = np.asarray(inputs["bo"], np.float32)

    xT = np.ascontiguousarray(x.reshape(BS, E).T).astype(bf16)
    xPk = _pack_x(xT)
    tri = _tri_np()
    onesk = np.ones((128, 128), np.float32)

    in_maps = []
    for c in range(NCORES):
        dsl = slice(c * DC, (c + 1) * DC)
        in_maps.append(
            {
                "xP": xPk,
                "wqP": _pack_w((Wq[dsl].T * SCALE).astype(bf16)),
                "wkP": _pack_w(Wk[dsl].T.astype(bf16)),
                "wvP": _pack_w(Wv[dsl].T.astype(bf16)),
                "woP": _pack_w(Wo[:, dsl].T.astype(bf16)),
                "bq": np.ascontiguousarray((bq[dsl] * SCALE).reshape(DC, 1)),
                "bk": np.ascontiguousarray(bk[dsl].reshape(DC, 1)),
                "tri": tri,
                "onesk": onesk,
            }
        )

    if _NC_CACHE is None:
        _NC_CACHE = _build()
    res = run_bass_kernel_spmd(_NC_CACHE, in_maps, core_ids=list(range(NCORES)), trace=TRACE)
    LAST_RESULTS = res

    acc = None
    for r in res.results:
        yc = np.asarray(r["y"], np.float32)
        acc = yc if acc is None else acc + yc
    bo_eff = bo + bv @ Wo.T
    acc += bo_eff[None, :]
    return acc.reshape(B, S, E).astype(np.float32)
